# revision 67
# baseline (speedup 1.0000x reference)
"""CBAM (channel attention + non-local spatial attention) Trainium2 kernel.

Full-input contract: kernel(**inputs) takes the complete tensors as produced by
setup_inputs() and returns the full [8, 256, 64, 64] output. Internally the
batch dimension (8) is sharded 1:1 across 8 NeuronCores; every core holds the
full (tiny) weights.

Per-core math (batch b, x viewed as [C=256, HW=4096]):
  gate  = sigmoid(W2 relu(W1 avg) + W2 relu(W1 max))          (channel attn)
  q,k   = (Wq*gate)/(Wk*gate) @ x + b                         [16, HW] bf16
  vT    = gamma * (x^T @ (Wv*gate)^T + v_b)                   [HW, 256] fp8
  S^T   = k^T q                                               [j, i] layout
  P     = exp(S^T - 2)         (constant shift cancels in softmax; keeps
                                exp() under fp8e4m3's 240 max)
  out2  = P^T @ [vT | 1]       -> [i, 256 cols of gamma*V@attn | denom]
  y     = transpose(out2[:, :256] / denom) + x*gate

The channel gate is folded into the 1x1 conv weights, so projections run on
the ungated bf16 x copy and the f32 x*gate product is only needed for the
final residual add. The value/probability operands are fp8e4m3 so the big
P^T @ vT matmul runs in DoubleRow mode (two 128-row j-tiles per instruction).
Numpy-emulated end-to-end precision of this mix: rel err ~8e-5 (tol 2e-2).
"""

import numpy as np
from contextlib import ExitStack

import concourse.bass as bass
import concourse.tile as tile
from concourse import bacc, mybir
from concourse.bass_utils import run_bass_kernel_spmd
from concourse.masks import make_identity

B, C, H, W = 8, 256, 64, 64
HW = H * W            # 4096
R = 16                # C // reduction
P = 128               # SBUF partitions
CT = C // P           # 2 channel tiles
NJ = HW // P          # 32 key tiles
NP = NJ // 2          # 16 key-tile pairs
SI = 512              # query super-block
NI = HW // SI         # 8 super-blocks
SHIFT = 2.0           # exp(s - SHIFT); cancels in softmax
F32 = mybir.dt.float32
BF16 = mybir.dt.bfloat16
F8 = mybir.dt.float8e4

AF = mybir.ActivationFunctionType
DR = mybir.MatmulPerfMode.DoubleRow


def _build_nc():
    nc = bacc.Bacc("TRN2", target_bir_lowering=False)
    x_d = nc.declare_dram_parameter("x", [C, HW], F32, isOutput=False)
    w1t_d = nc.declare_dram_parameter("w1t", [P, CT * R], F32, isOutput=False)
    w2t_d = nc.declare_dram_parameter("w2t", [R, C], F32, isOutput=False)
    qwt_d = nc.declare_dram_parameter("qwt", [P, CT * R], F32, isOutput=False)
    kwt_d = nc.declare_dram_parameter("kwt", [P, CT * R], F32, isOutput=False)
    vwt_d = nc.declare_dram_parameter("vwt", [P, CT * C], F32, isOutput=False)
    qb_d = nc.declare_dram_parameter("qb", [R, 1], F32, isOutput=False)
    kb_d = nc.declare_dram_parameter("kb", [R, 1], F32, isOutput=False)
    vb_d = nc.declare_dram_parameter("vb", [1, C], F32, isOutput=False)
    g_d = nc.declare_dram_parameter("gamma", [1, 1], F32, isOutput=False)
    y_d = nc.declare_dram_parameter("y", [C, HW], F32, isOutput=True)

    with ExitStack() as ctx:
        tc = ctx.enter_context(tile.TileContext(nc))
        const = ctx.enter_context(tc.tile_pool(name="const", bufs=1))
        xp = ctx.enter_context(tc.tile_pool(name="xp", bufs=1))
        xbp = ctx.enter_context(tc.tile_pool(name="xbp", bufs=1))
        qkp = ctx.enter_context(tc.tile_pool(name="qkp", bufs=1))
        vtp = ctx.enter_context(tc.tile_pool(name="vtp", bufs=1))
        pp = ctx.enter_context(tc.tile_pool(name="pp", bufs=2))
        small = ctx.enter_context(tc.tile_pool(name="small", bufs=4))
        outp = ctx.enter_context(tc.tile_pool(name="outp", bufs=2))
        # PSUM banks: sc 2x2 + o 2x1 + tp 2x1 = 8
        ps_s = ctx.enter_context(tc.tile_pool(name="ps_s", bufs=2, space="PSUM"))
        ps_o = ctx.enter_context(tc.tile_pool(name="ps_o", bufs=2, space="PSUM"))
        ps_t = ctx.enter_context(tc.tile_pool(name="ps_t", bufs=2, space="PSUM"))

        # ---------------- loads ----------------
        # x loads: HBM-bandwidth bound (~12us); shorter trailing chunks so
        # the reduce/gate chain hanging off the last chunk starts sooner
        x_chunks = [
            [(0, 1024), (1024, 2048), (2048, 3072), (3072, 3584), (3584, 4096)],
            [(0, 1024), (1024, 2048), (2048, 3072), (3072, 3584), (3584, 4096)],
        ]
        x_q = [nc.sync, nc.scalar, nc.gpsimd]
        x_sb = []
        qi = 0
        for ct in range(CT):
            xt = xp.tile([P, HW], F32, name=f"x{ct}", tag=f"x{ct}")
            for lo, hi in x_chunks[ct]:
                x_q[qi % 3].dma_start(out=xt[:, lo:hi],
                                      in_=x_d[ct * P:(ct + 1) * P, lo:hi])
                qi += 1
            x_sb.append(xt)

        w1t = const.tile([P, CT * R], F32, name="w1t")
        nc.sync.dma_start(out=w1t, in_=w1t_d[:, :])
        w2t = const.tile([R, C], F32, name="w2t")
        nc.sync.dma_start(out=w2t, in_=w2t_d[:, :])
        qwt = const.tile([P, CT * R], F32, name="qwt")
        nc.sync.dma_start(out=qwt, in_=qwt_d[:, :])
        kwt = const.tile([P, CT * R], F32, name="kwt")
        nc.sync.dma_start(out=kwt, in_=kwt_d[:, :])
        vwt = const.tile([P, CT * C], F32, name="vwt")
        nc.sync.dma_start(out=vwt, in_=vwt_d[:, :])
        qb = const.tile([R, 1], F32, name="qb")
        nc.sync.dma_start(out=qb, in_=qb_d[:, :])
        kb = const.tile([R, 1], F32, name="kb")
        nc.sync.dma_start(out=kb, in_=kb_d[:, :])
        vb = const.tile([1, C], F32, name="vb")
        nc.sync.dma_start(out=vb, in_=vb_d[:, :])
        g128 = const.tile([P, 1], F32, name="g128")
        nc.gpsimd.dma_start(out=g128, in_=g_d[:, :].to_broadcast([P, 1]))

        ident = const.tile([P, P], BF16, name="ident")
        make_identity(nc, ident)
        ones1 = const.tile([1, P], BF16, name="ones1")
        nc.vector.memset(ones1, 1.0)
        # NOTE: the vb -> vb_bf cast is emitted after the gate fold, not
        # here: vb's DMA lands last on the sync queue (~16us) and an early
        # cast would block the in-order DVE pipeline right when the
        # channel-attention reduces need it.
        vb_bf = const.tile([1, C], BF16, name="vb_bf")
        # [1, 0] per (pair-subtile, col): columns 256/257 of the vt tiles
        # (ones -> softmax denominator; zero pad -> even moving dim)
        onescols = const.tile([P, 2, 2], F8, name="onescols")
        nc.vector.memset(onescols[:, :, 0:1], 1.0)
        nc.vector.memset(onescols[:, :, 1:2], 0.0)
        nshift = const.tile([P, 1], F32, name="nshift")
        nc.vector.memset(nshift, -SHIFT)

        # ---------------- channel attention + bf16 cast ----------------
        # The bf16 cast of x doubles as the sum-reduce pass (ACT accum_out);
        # the max-reduce runs chunked on DVE chasing the x DMA chunks.
        x_bf = []
        sums = []
        maxs = []
        for ct in range(CT):
            nch = len(x_chunks[ct])
            xb = xbp.tile([P, HW], BF16, name=f"xb{ct}", tag=f"xb{ct}")
            sm = small.tile([P, nch], F32, name=f"sm{ct}", tag=f"sm{ct}", bufs=1)
            mx = small.tile([P, nch], F32, name=f"mx{ct}", tag=f"mx{ct}", bufs=1)
            for ch, (lo, hi) in enumerate(x_chunks[ct]):
                sl = slice(lo, hi)
                nc.scalar.activation(out=xb[:, sl], in_=x_sb[ct][:, sl],
                                     func=AF.Copy, accum_out=sm[:, ch:ch + 1])
                nc.vector.reduce_max(out=mx[:, ch:ch + 1], in_=x_sb[ct][:, sl],
                                     axis=mybir.AxisListType.X)
            x_bf.append(xb)
            sums.append(sm)
            maxs.append(mx)

        s_av = []
        for ct in range(CT):
            sa = small.tile([P, 2], F32, name=f"sa{ct}", tag=f"sa{ct}", bufs=1)
            nc.vector.reduce_sum(out=sa[:, 0:1], in_=sums[ct], axis=mybir.AxisListType.X)
            nc.vector.tensor_scalar_mul(out=sa[:, 0:1], in0=sa[:, 0:1], scalar1=1.0 / HW)
            nc.vector.reduce_max(out=sa[:, 1:2], in_=maxs[ct], axis=mybir.AxisListType.X)
            s_av.append(sa)

        h_ps = ps_s.tile([R, 2], F32, name="h_ps", tag="sc_ps")
        for ct in range(CT):
            nc.tensor.matmul(h_ps, lhsT=w1t[:, ct * R:(ct + 1) * R], rhs=s_av[ct],
                             start=(ct == 0), stop=(ct == CT - 1))
        h_sb = small.tile([R, 2], F32, name="h_sb")
        nc.scalar.activation(out=h_sb, in_=h_ps, func=AF.Relu)

        # sigmoid(z) = 0.5*tanh(z/2) + 0.5 -- tanh shares the exp table set.
        # The avg+max sum rides the ACT accumulator: a DVE tensor_add with
        # both operands in the same PSUM tile fails neuronxcc compilation
        # (two hardware-verified repros), so keep this on ACT.
        zts = small.tile([P, CT], F32, name="zts", tag="zts", bufs=1)
        gtrash = small.tile([P, 2], F32, name="gtrash")
        for ct in range(CT):
            g_ps = ps_o.tile([P, 2], F32, name="g_ps", tag="o_ps")
            nc.tensor.matmul(g_ps, lhsT=w2t[:, ct * P:(ct + 1) * P], rhs=h_sb,
                             start=True, stop=True)
            nc.scalar.activation(out=gtrash, in_=g_ps, func=AF.Identity,
                                 accum_out=zts[:, ct:ct + 1])
        th2 = small.tile([P, CT], F32, name="th2")
        nc.scalar.activation(out=th2, in_=zts, func=AF.Tanh, scale=0.5)
        gates2 = small.tile([P, CT], F32, name="gates2", tag="gates2", bufs=1)
        nc.vector.tensor_scalar(out=gates2, in0=th2, scalar1=0.5, scalar2=0.5,
                                op0=mybir.AluOpType.mult,
                                op1=mybir.AluOpType.add)
        gates = [gates2[:, ct:ct + 1] for ct in range(CT)]

        # ---------------- gate-folded bf16 weights ----------------
        qwt_bf = const.tile([P, CT * R], BF16, name="qwt_bf")
        kwt_bf = const.tile([P, CT * R], BF16, name="kwt_bf")
        vwt_bf = const.tile([P, CT * C], BF16, name="vwt_bf")
        for ct in range(CT):
            rsl = slice(ct * R, (ct + 1) * R)
            csl = slice(ct * C, (ct + 1) * C)
            nc.vector.tensor_scalar_mul(out=qwt_bf[:, rsl], in0=qwt[:, rsl],
                                        scalar1=gates[ct])
            nc.vector.tensor_scalar_mul(out=kwt_bf[:, rsl], in0=kwt[:, rsl],
                                        scalar1=gates[ct])
            nc.vector.tensor_scalar_mul(out=vwt_bf[:, csl], in0=vwt[:, csl],
                                        scalar1=gates[ct])
        # on Pool: vb's DMA lands last on the sync queue, and even a parked
        # DVE copy waiting on it delays later DVE dispatches in the queue
        nc.gpsimd.tensor_copy(out=vb_bf, in_=vb)

        # ---------------- q, k projections ----------------
        q_sb = qkp.tile([R, HW], BF16, name="q_sb", tag="q_sb")
        k_sb = qkp.tile([R, HW], BF16, name="k_sb", tag="k_sb")
        # ---------------- score pairs ----------------
        # j-tiles (2p, 2p+1) computed by two K=16 matmuls into one 2-bank
        # PSUM tile; a single exp covers both and writes the fp8
        # probability pair directly in DoubleRow [128, 2, 512] layout.
        def emit_score_pair(si, pr):
            sc = ps_s.tile([P, 2, SI], F32, name="sc_ps", tag="sc_ps")
            for g in range(2):
                jb = 2 * pr + g
                nc.tensor.matmul(
                    sc[:, g, :],
                    lhsT=k_sb[:, jb * P:(jb + 1) * P],
                    rhs=q_sb[:, si * SI:(si + 1) * SI],
                    start=True, stop=True)
            pt = pp.tile([P, 2, SI], F8, name=f"pt{pr}", tag=f"pt{pr}")
            nc.scalar.activation(out=pt, in_=sc, func=AF.Exp, bias=nshift)
            return pt

        def build_vt(pr):
            vt = vtp.tile([P, 2, C + 2], F8, name=f"vt{pr}", tag=f"vt{pr}")
            # alternate psum banks so four builds are in flight; otherwise
            # the 2-buf rotation paces the chain at one DVE scale per pair
            ps = ps_o if pr % 2 == 0 else ps_t
            tag = "o_ps" if pr % 2 == 0 else "tp_ps"
            vt_ps = ps.tile([P, 2, C], F32, name="vt_ps", tag=tag)
            for g in range(2):
                jb = 2 * pr + g
                for ct in range(CT):
                    nc.tensor.matmul(
                        vt_ps[:, g, :], lhsT=x_bf[ct][:, jb * P:(jb + 1) * P],
                        rhs=vwt_bf[:, ct * C:(ct + 1) * C],
                        start=(ct == 0), stop=False)
                nc.tensor.matmul(vt_ps[:, g, :], lhsT=ones1, rhs=vb_bf,
                                 start=False, stop=True)
            nc.vector.tensor_scalar_mul(out=vt[:, :, 0:C], in0=vt_ps,
                                        scalar1=g128)
            nc.gpsimd.tensor_copy(out=vt[:, :, C:C + 2], in_=onescols)
            return vt

        # residual x*gate in place, chunked si-major on the (otherwise idle)
        # Pool engine so DVE stays free; si=0's chunks first (the first y
        # adds need them), the rest after the prologue
        def xg_chunk(s):
            for ct in range(CT):
                sl = slice(s * SI, (s + 1) * SI)
                nc.gpsimd.tensor_scalar_mul(out=x_sb[ct][:, sl],
                                            in0=x_sb[ct][:, sl],
                                            scalar1=gates[ct])

        xg_chunk(0)

        # Projections in per-ib [16,512] psum tiles from the tp_ps bank
        # (idle until the main loop) so they never contend with the score
        # pairs' sc_ps rotation. Score pairs are interleaved between the
        # proj ib-groups they depend on, so the exp stream starts as soon
        # as ib0/ib1 are done.
        def proj_ib(ib, sides):
            for dst, wt, bias in sides:
                pj = ps_t.tile([R, 512], F32, name="pj", tag="tp_ps")
                for ct in range(CT):
                    nc.tensor.matmul(
                        pj, lhsT=wt[:, ct * R:(ct + 1) * R],
                        rhs=x_bf[ct][:, ib * 512:(ib + 1) * 512],
                        start=(ct == 0), stop=(ct == CT - 1))
                cols = slice(ib * 512, (ib + 1) * 512)
                if ib == 0 and dst is k_sb:
                    # parallel to q0's DVE add; ACT is idle pre-exp-stream
                    nc.scalar.activation(out=dst[0:R, cols], in_=pj,
                                         func=AF.Identity, bias=bias)
                else:
                    nc.vector.tensor_scalar_add(out=dst[0:R, cols], in0=pj,
                                                scalar1=bias)

        QSIDE = ((q_sb, qwt_bf, qb),)
        KSIDE = ((k_sb, kwt_bf, kb),)
        vt_pairs = [None] * NP
        cur = [None] * NP
        for ib in range(HW // 512):
            # q columns beyond ib1 are first read at si=2, ~27us into the
            # exp stream -- defer them past the vt builds so the DVE
            # prologue chain stays short
            proj_ib(ib, QSIDE + KSIDE if ib < 2 else KSIDE)
            # score pair p reads k columns entirely within ib p//2, so two
            # pairs follow each ib immediately -- the first exp fires right
            # after ib0 instead of after ib1
            for pr in (2 * ib, 2 * ib + 1):
                cur[pr] = emit_score_pair(0, pr)
            # one vt build per ib: enough DVE slack that the k bias-adds of
            # later ibs aren't pushed past their score pairs
            vt_pairs[ib] = build_vt(ib)
        # si=1's first half of score pairs go ahead of the remaining vt
        # builds, so the 48-matmul vt wall can't separate si=0's exps from
        # si=1's on the in-order PE queue
        nxt0 = [None] * NP
        for half in range(2):
            for pr in range(4 * half, 4 * half + 4):
                nxt0[pr] = emit_score_pair(1, pr)
            for pr in range(NP // 2 + 4 * half, NP // 2 + 4 * half + 4):
                vt_pairs[pr] = build_vt(pr)
        for s in range(1, NI):
            xg_chunk(s)

        # ---------------- spatial attention main loop ----------------
        y_last = [outp.tile([P, SI], F32, name=f"ylast{ct}", tag=f"ylast{ct}",
                            bufs=1) for ct in range(CT)]
        for si in range(NI):
            nxt = nxt0 if si == 0 else [None] * NP
            # q columns for super-block si+2 are first read by the score
            # pairs emitted during si+1 -- projecting them here keeps the
            # prologue's DVE chain short
            if si + 2 < NI:
                proj_ib(si + 2, QSIDE)
            tp_ps = ps_t.tile([P, CT, SI], BF16, name="tp_ps", tag="tp_ps")
            for ii in range(SI // P):
                i0 = si * SI + ii * P
                # scores for the next super-block go first: they have no vt
                # dependency, so a vt-stalled PV block can't head-of-line
                # block the exp stream. si=0 already carries si=1's first
                # half from the prologue, so it emits the second half.
                if si + 1 < NI:
                    if si == 0:
                        prs = range(8 + 4 * ii, 12 + 4 * ii) if ii < 2 else ()
                    else:
                        prs = range(4 * ii, 4 * ii + 4)
                    for t in prs:
                        nxt[t] = emit_score_pair(si + 1, t)
                o_ps = ps_o.tile([P, C + 2], F32, name="o_ps", tag="o_ps")
                for pr in range(NP):
                    nc.tensor.matmul(
                        o_ps, lhsT=cur[pr][:, :, ii * P:(ii + 1) * P],
                        rhs=vt_pairs[pr],
                        start=(pr == 0), stop=(pr == NP - 1),
                        perf_mode=DR)
                rec = small.tile([P, 1], F32, name="rec")
                nc.vector.reciprocal(out=rec, in_=o_ps[:, C:C + 1])
                r2 = outp.tile([P, C], BF16, name="r2", bufs=3)
                nc.vector.tensor_scalar_mul(out=r2, in0=o_ps[:, 0:C], scalar1=rec)
                for ct in range(CT):
                    nc.tensor.transpose(tp_ps[:, ct, ii * P:(ii + 1) * P],
                                        r2[:, ct * P:(ct + 1) * P], ident)
                if si == NI - 1:
                    # last super-block: per-ii adds into the staging tiles
                    # so only the last slice's add + one DMA per ct remain
                    # after the final transpose
                    for ct in range(CT):
                        nc.vector.tensor_add(
                            out=y_last[ct][:, ii * P:(ii + 1) * P],
                            in0=tp_ps[:, ct, ii * P:(ii + 1) * P],
                            in1=x_sb[ct][:, i0:i0 + P])
            if si < NI - 1:
                for ct in range(CT):
                    y_sb = outp.tile([P, SI], F32, name="y_sb", bufs=4)
                    nc.vector.tensor_add(out=y_sb, in0=tp_ps[:, ct, :],
                                         in1=x_sb[ct][:, si * SI:(si + 1) * SI])
                    nc.sync.dma_start(
                        out=y_d[ct * P:(ct + 1) * P, si * SI:(si + 1) * SI],
                        in_=y_sb)
            else:
                # halves on separate queues: the first half flies while the
                # last transposes finish, only a [128,256] store trails
                for half in range(2):
                    cols = slice(half * SI // 2, (half + 1) * SI // 2)
                    for ct, q in zip(range(CT), (nc.scalar, nc.sync)):
                        q.dma_start(
                            out=y_d[ct * P:(ct + 1) * P,
                                    si * SI + half * SI // 2:
                                    si * SI + (half + 1) * SI // 2],
                            in_=y_last[ct][:, cols])
            cur = nxt
    nc.compile()
    return nc


_NC_CACHE = {}


def _get_nc():
    if "nc" not in _NC_CACHE:
        _NC_CACHE["nc"] = _build_nc()
    return _NC_CACHE["nc"]


def _make_in_maps(inputs):
    x = np.ascontiguousarray(np.asarray(inputs["x"], dtype=np.float32))
    ca_w1 = np.asarray(inputs["ca_w1"], np.float32)
    ca_w2 = np.asarray(inputs["ca_w2"], np.float32)
    q_w = np.asarray(inputs["q_w"], np.float32)
    q_b = np.asarray(inputs["q_b"], np.float32)
    k_w = np.asarray(inputs["k_w"], np.float32)
    k_b = np.asarray(inputs["k_b"], np.float32)
    v_w = np.asarray(inputs["v_w"], np.float32)
    v_b = np.asarray(inputs["v_b"], np.float32)
    gamma = np.asarray(inputs["gamma"], np.float32)

    def ktiles(wT):  # [C, n] -> [128, CT*n] with c-tile-major free dim
        n = wT.shape[1]
        return np.ascontiguousarray(
            wT.reshape(CT, P, n).transpose(1, 0, 2).reshape(P, CT * n))

    shared = {
        "w1t": ktiles(ca_w1.T.copy()),
        "w2t": np.ascontiguousarray(ca_w2.T),
        "qwt": ktiles(q_w.T.copy()),
        "kwt": ktiles(k_w.T.copy()),
        "vwt": ktiles(v_w.T.copy()),
        "qb": q_b.reshape(R, 1).copy(),
        "kb": k_b.reshape(R, 1).copy(),
        "vb": v_b.reshape(1, C).copy(),
        "gamma": gamma.reshape(1, 1).copy(),
    }
    return [{"x": x[b].reshape(C, HW).copy(), **shared} for b in range(B)]


def _run(inputs, trace=False):
    nc = _get_nc()
    in_maps = _make_in_maps(inputs)
    bkr = run_bass_kernel_spmd(nc, in_maps, list(range(B)), trace=trace)
    out = np.stack([np.asarray(bkr.results[b]["y"]).reshape(C, H, W)
                    for b in range(B)])
    return out, bkr


def kernel(**inputs) -> np.ndarray:
    out, _ = _run(inputs, trace=False)
    return out
